# revision 8
# baseline (speedup 1.0000x reference)
"""Trainium2 Bass kernel for nn_EnhancedGNNTransformerEncoder (4-layer
TransformerConv GNN + mean-pool + linear head).

Sharding: destination nodes (and their incident edges) are distributed
round-robin by degree across the 8 NeuronCores; the small weight matrices are
replicated.  Per layer each core computes K/V (fp8) + Q/skip (fp16) rows for
its own shard, the fp8 K|V rows are exchanged with an AllGather, and each core
runs the per-edge attention for its own destination windows.

Edge layout is *lane-aligned*: window w holds 128 destination nodes, one per
SBUF partition lane; slot s of lane p holds the s-th incoming edge of that
lane's node (split into a lo/hi region by source half for int16 gather
indices).  Per-edge K|V rows are fetched with dma_gather; q / softmax /
weighted aggregation then need no per-edge index work at all: q is a
contiguous row load and the segment sum over slots is an identity matmul
accumulated in PSUM.
"""

import os
import sys
import types

import numpy as np

# ---------------------------------------------------------------------------
# NTFF profile hook (absent antenv.axon_hooks on this image) so trace=True
# works under axon.
if "antenv.axon_hooks" not in sys.modules:
    _m = types.ModuleType("antenv.axon_hooks")
    _m._hook = None

    def _set(h):
        _m._hook = h

    def _get():
        return _m._hook

    _m.set_axon_ntff_profile_hook = _set
    _m.get_axon_ntff_profile_hook = _get
    sys.modules["antenv.axon_hooks"] = _m
    try:
        import antenv

        antenv.axon_hooks = _m
    except Exception:
        pass
    try:
        from trn_agent_boot.trn_boot import _ntff_profile_via_ctypes

        _m._hook = _ntff_profile_via_ctypes("/opt/axon/libaxon_pjrt.so")
    except Exception:
        pass

import concourse.bass as bass
import concourse.mybir as mybir
import concourse.tile as tile
from concourse import bacc
from concourse import bass_utils
from concourse.masks import make_identity

F16 = mybir.dt.float16
BF16 = mybir.dt.bfloat16
F32 = mybir.dt.float32
F8 = mybir.dt.float8e4
I16 = mybir.dt.int16
AX = mybir.AxisListType
OP = mybir.AluOpType
ACTF = mybir.ActivationFunctionType

# problem constants (hardcoded per the harness contract)
N, E, IN, H, C, G, OUT = 50000, 800000, 128, 8, 32, 64, 64
HC = H * C  # 256
NLAYERS = 4
NC = 8
NLOC = N // NC          # 6250
W = 49                  # windows of 128 dst lanes per core
NPAD = W * 128          # 6272 local rows (incl dummies)
NPAD_ALL = NC * NPAD    # 50176 kv rows
HALF = NPAD_ALL // 2    # 25088 (int16-addressable half; cores 0-3)
SCALE = float(1.0 / np.sqrt(C))
MASK_NEG = -30000.0
SCHUNK = 16             # slots per processing chunk

_BUILD_CACHE = {}
LAST_RESULT = None


def _build(SL, SH, run_layers, use_bias, kv8):
    """Build + compile the SPMD program.  SL/SH: per-window lo/hi slot counts
    (uniform across cores)."""
    key = (tuple(SL), tuple(SH), run_layers, use_bias, kv8, int(os.environ.get('KSCHUNK', str(SCHUNK))))
    if key in _BUILD_CACHE:
        return _BUILD_CACHE[key]

    SLOTS = int(sum(SL) + sum(SH))      # total kv slots per core
    IDXC = SLOTS * 8                    # idx columns (128 i16 per slot = 8 cols)
    KVD = F8 if kv8 else F16
    nwin = int(os.environ.get("KWIN", str(W)))
    schunk = int(os.environ.get("KSCHUNK", str(SCHUNK)))

    # per-window chunk lists: (half, slot_off_in_window, csz)
    chunks = []
    for w in range(W):
        ch = []
        for a in range(0, SL[w], schunk):
            ch.append((0, a, min(schunk, SL[w] - a)))
        for a in range(0, SH[w], schunk):
            ch.append((1, a, min(schunk, SH[w] - a)))
        chunks.append(ch)
    woff = np.concatenate([[0], np.cumsum(np.asarray(SL) + np.asarray(SH))])

    nc = bacc.Bacc("TRN2", target_bir_lowering=False, debug=False,
                   enable_asserts=False, num_devices=NC,
                   num_swdge_queues=4)

    # ---- external inputs (per-core content, same shapes) ----
    xT_loc = nc.dram_tensor("xT_loc", [128, NPAD], F16, kind="ExternalInput")
    w_all = nc.dram_tensor("w_all", [128, NLAYERS, 2, 4 * HC], F16, kind="ExternalInput")
    b_all = nc.dram_tensor("b_all", [1, NLAYERS, 4 * HC], F16, kind="ExternalInput")
    ones1 = nc.dram_tensor("ones1", [1, 128], F16, kind="ExternalInput")
    w_fc = nc.dram_tensor("w_fc", [128, 2, OUT], F16, kind="ExternalInput")
    b_fc = nc.dram_tensor("b_fc", [G, OUT], F32, kind="ExternalInput")
    idx_all = nc.dram_tensor("idx_all", [128, IDXC], I16, kind="ExternalInput")
    maskw = nc.dram_tensor("maskw", [128, SLOTS], F16, kind="ExternalInput")
    gsel = nc.dram_tensor("gsel", [128, W * G], F16, kind="ExternalInput")

    out_d = nc.dram_tensor("out", [G, OUT], F32, kind="ExternalOutput")

    # ---- internal DRAM (ping-pong by layer parity) ----
    kv_loc = [nc.dram_tensor(f"kv_loc{i}", [NPAD, 2 * HC], KVD, kind="Internal")
              for i in range(2)]
    kv_full = [nc.dram_tensor(f"kv_full{i}", [NPAD_ALL, 2 * HC], KVD,
                              kind="Internal", addr_space="Shared")
               for i in range(2)]
    qskip = [nc.dram_tensor(f"qskip{i}", [NPAD, 2 * HC], F16, kind="Internal")
             for i in range(2)]
    pool_part = nc.dram_tensor("pool_part", [2, 128, OUT], F32, kind="Internal")
    pool_sum = nc.dram_tensor("pool_sum", [2, 128, OUT], F32, kind="Internal",
                              addr_space="Shared")

    GRP = 7              # node-tiles per phase-A store group (49 = 7*7)
    NGRP = W // GRP

    with tile.TileContext(nc) as tc:
        with tc.tile_pool(name="const", bufs=1) as cp, \
             tc.tile_pool(name="kvb", bufs=2) as kvbp, \
             tc.tile_pool(name="win", bufs=2) as winp, \
             tc.tile_pool(name="psA", bufs=2, space="PSUM") as psA, \
             tc.tile_pool(name="psB", bufs=2, space="PSUM") as psB, \
             tc.tile_pool(name="psT", bufs=2, space="PSUM") as psT:

            # ---- load constants ----
            wall_sb = cp.tile([128, NLAYERS, 2, 4 * HC], F16)
            ball_sb = cp.tile([1, NLAYERS, 4 * HC], F16)
            ones_sb = cp.tile([1, 128], F16)
            wfc_sb = cp.tile([128, 2, OUT], F16)
            bfc_sb = cp.tile([G, OUT], F32)
            mask_sb = cp.tile([128, SLOTS], F16)
            gsel_sb = cp.tile([128, W * G], F16)
            ident16 = cp.tile([128, 128], F16)
            identbf = cp.tile([128, 128], BF16)
            pool_acc = cp.tile([128, 2, OUT], F32)
            hT_sb = cp.tile([128, 2, NPAD], F16)

            for t, d in [(wall_sb, w_all), (ball_sb, b_all),
                         (ones_sb, ones1), (wfc_sb, w_fc),
                         (bfc_sb, b_fc), (mask_sb, maskw), (gsel_sb, gsel)]:
                nc.sync.dma_start(out=t[:], in_=d.ap())
            make_identity(nc, ident16[:])
            make_identity(nc, identbf[:])
            nc.sync.dma_start(out=hT_sb[:, 0, :], in_=xT_loc.ap())

            qctr = [0]

            def a_group(layer, g):
                """Phase A for node tiles [g*GRP, (g+1)*GRP): K|V + Q|S rows."""
                KH = 1 if layer == 0 else 2
                par = layer % 2
                kvb = kvbp.tile([128, GRP, 2 * HC], KVD, tag="kvb")
                qsb = kvbp.tile([128, GRP, 2 * HC], F16, tag="qsb")
                for jg in range(GRP):
                    j = g * GRP + jg
                    ps = psA.tile([128, 4 * HC], F32, tag="psA")
                    for hf in range(2):
                        cs = slice(hf * 2 * HC, (hf + 1) * 2 * HC)
                        for kh in range(KH):
                            nc.tensor.matmul(
                                ps[:, cs],
                                lhsT=hT_sb[:, kh, j * 128:(j + 1) * 128],
                                rhs=wall_sb[:, layer, kh, cs],
                                start=(kh == 0),
                                stop=(not use_bias and kh == KH - 1),
                                skip_group_check=True)
                        if use_bias:
                            nc.tensor.matmul(
                                ps[:, cs], lhsT=ones_sb[:],
                                rhs=ball_sb[:, layer, cs],
                                start=False, stop=True,
                                skip_group_check=True)
                    if jg % 2 == 0:
                        nc.vector.tensor_copy(kvb[:, jg, :], ps[:, 0:2 * HC])
                        nc.scalar.activation(qsb[:, jg, :],
                                             ps[:, 2 * HC:4 * HC], ACTF.Copy)
                    else:
                        nc.scalar.activation(kvb[:, jg, :],
                                             ps[:, 0:2 * HC], ACTF.Copy)
                        nc.vector.tensor_copy(qsb[:, jg, :],
                                              ps[:, 2 * HC:4 * HC])
                base = g * GRP * 128
                dst_ap = kv_loc[par].ap()[base:base + GRP * 128, :]
                nc.sync.dma_start(
                    out=dst_ap.rearrange("(t p) e -> p t e", p=128),
                    in_=kvb[:])
                dst_ap = qskip[par].ap()[base:base + GRP * 128, :]
                nc.sync.dma_start(
                    out=dst_ap.rearrange("(t p) e -> p t e", p=128),
                    in_=qsb[:])

            def fire_collective(layer):
                par = layer % 2
                nc.gpsimd.collective_compute(
                    "AllGather", OP.bypass,
                    replica_groups=[list(range(NC))],
                    ins=[kv_loc[par].ap()], outs=[kv_full[par].ap()])

            # prologue: layer-0 phase A + collective
            for g in range(NGRP):
                a_group(0, g)
            fire_collective(0)

            for layer in range(run_layers):
                last = layer == NLAYERS - 1
                par = layer % 2

                # ===== Phase B: lane-aligned windows =====
                if last:
                    pacc = [psT.tile([128, OUT], F32, tag="trp",
                                     name=f"pacc{kh}") for kh in range(2)]
                for w in range(nwin):
                    S = SL[w] + SH[w]
                    wchunks = chunks[w]
                    idxw = winp.tile([128, S * 8], I16, tag="idxw", bufs=3)
                    nc.sync.dma_start(
                        out=idxw[:],
                        in_=idx_all.ap()[:, woff[w] * 8:(woff[w] + S) * 8])
                    qs_w = winp.tile([128, 2 * HC], F16, tag="qsw", bufs=2)
                    nc.sync.dma_start(
                        out=qs_w[:],
                        in_=qskip[par].ap()[w * 128:(w + 1) * 128, :])
                    agg = psB.tile([128, HC + H], F32, tag="agg")

                    for ci, (half, a, csz) in enumerate(wchunks):
                        # slot offset of this chunk within the window
                        soff = a if half == 0 else SL[w] + a
                        kv8_t = winp.tile([128, schunk, 2 * HC], KVD,
                                          tag="kv8", bufs=3)
                        in_ap = (kv_full[par].ap()[0:HALF, :] if half == 0
                                 else kv_full[par].ap()[HALF:NPAD_ALL, :])
                        for b in range(0, csz, 8):
                            bsz = min(8, csz - b)
                            nc.gpsimd.dma_gather(
                                out_ap=kv8_t[:, b:b + bsz, :],
                                in_ap=in_ap,
                                idxs_ap=idxw[:, (soff + b) * 8:
                                             (soff + b + bsz) * 8],
                                num_idxs=bsz * 128, num_idxs_reg=bsz * 128,
                                elem_size=2 * HC, single_packet=True,
                                queue_num=qctr[0] % 4)
                            qctr[0] += 1
                        if kv8:
                            kvb_t = winp.tile([128, schunk, 2 * HC], F16,
                                              tag="kvc", bufs=2)
                            nc.scalar.activation(kvb_t[:, 0:csz, :],
                                                 kv8_t[:, 0:csz, :], ACTF.Copy)
                        else:
                            kvb_t = kv8_t
                        # logits
                        qk = winp.tile([128, schunk, HC], F16, tag="qkpx", bufs=2)
                        nc.vector.tensor_tensor(
                            qk[:, 0:csz, :], kvb_t[:, 0:csz, 0:HC],
                            qs_w[:, None, 0:HC].to_broadcast((128, csz, HC)),
                            OP.mult)
                        lg = winp.tile([128, schunk * H], F16, tag="lg", bufs=2)
                        with nc.allow_low_precision(reason="logits f16 for 2x DVE"):
                            nc.vector.tensor_reduce(
                                lg[:, 0:csz * H],
                                qk[:, 0:csz, :].rearrange(
                                    "p s (h c) -> p (s h) c", c=C),
                                axis=AX.X, op=OP.add)
                        ml = winp.tile([128, schunk * H], F16, tag="ml", bufs=2)
                        nc.vector.scalar_tensor_tensor(
                            out=ml[:, 0:csz * H].rearrange(
                                "p (s h) -> p s h", h=H),
                            in0=lg[:, 0:csz * H].rearrange(
                                "p (s h) -> p s h", h=H),
                            scalar=SCALE,
                            in1=mask_sb[:, woff[w] + soff:woff[w] + soff + csz,
                                        None].to_broadcast((128, csz, H)),
                            op0=OP.mult, op1=OP.add)
                        # p = exp(ml), written into the extra H columns of wv
                        wv = winp.tile([128, schunk, HC + H], BF16, tag="wv",
                                       bufs=2)
                        nc.scalar.activation(
                            wv[:, 0:csz, HC:HC + H],
                            ml[:, 0:csz * H].rearrange("p (s h) -> p s h", h=H),
                            ACTF.Exp)
                        pX = winp.tile([128, schunk, HC], BF16, tag="qkpx",
                                       bufs=2)
                        nc.scalar.activation(
                            pX[:, 0:csz, :].rearrange(
                                "p s (h c) -> p s h c", c=C),
                            wv[:, 0:csz, HC:HC + H][:, :, :, None]
                            .to_broadcast((128, csz, H, C)),
                            ACTF.Copy)
                        nc.vector.tensor_tensor(
                            wv[:, 0:csz, 0:HC], kvb_t[:, 0:csz, HC:2 * HC],
                            pX[:, 0:csz, :], OP.mult)
                        for s in range(csz):
                            nc.tensor.matmul(
                                agg[:], lhsT=identbf[:], rhs=wv[:, s, :],
                                start=(ci == 0 and s == 0),
                                stop=(ci == len(wchunks) - 1 and s == csz - 1),
                                skip_group_check=True)

                    # ---- epilogue (once per window) ----
                    rs0 = winp.tile([128, H], F32, tag="rs0", bufs=2)
                    nc.vector.tensor_scalar_add(rs0[:], agg[:, HC:HC + H], 1e-16)
                    rs = winp.tile([128, H], F32, tag="rs", bufs=2)
                    nc.vector.reciprocal(rs[:], rs0[:])
                    tmp = winp.tile([128, HC], F32, tag="tmp", bufs=2)
                    nc.vector.tensor_tensor(
                        tmp[:].rearrange("p (h c) -> p h c", c=C),
                        agg[:, 0:HC].rearrange("p (h c) -> p h c", c=C),
                        rs[:, :, None].to_broadcast((128, H, C)),
                        OP.mult)
                    tmp2 = winp.tile([128, HC], F32, tag="tmp2", bufs=2)
                    nc.vector.tensor_tensor(tmp2[:], tmp[:], qs_w[:, HC:2 * HC],
                                            OP.add)
                    h_nm = winp.tile([128, HC], F16, tag="hnm", bufs=2)
                    nc.scalar.activation(h_nm[:], tmp2[:], ACTF.Relu)

                    if last:
                        for kh in range(2):
                            nc.tensor.matmul(
                                pacc[kh][:],
                                lhsT=h_nm[:, kh * 128:(kh + 1) * 128],
                                rhs=gsel_sb[:, w * G:(w + 1) * G],
                                start=(w == 0), stop=(w == nwin - 1),
                                skip_group_check=True)
                    else:
                        for kh in range(2):
                            trp = psT.tile([128, 128], F16, tag="trp")
                            nc.tensor.transpose(
                                trp[:], h_nm[:, kh * 128:(kh + 1) * 128],
                                ident16[:])
                            nc.scalar.activation(
                                hT_sb[:, kh, w * 128:(w + 1) * 128], trp[:],
                                ACTF.Copy)

                    # interleave next layer's phase A + collective
                    if not last:
                        if w % GRP == GRP - 1 and w // GRP < NGRP:
                            a_group(layer + 1, w // GRP)
                        if w == nwin - 1:
                            fire_collective(layer + 1)

                # ===== Phase C =====
                if last:
                    nc.vector.tensor_copy(pool_acc[:, 0, :], pacc[0][:])
                    nc.scalar.activation(pool_acc[:, 1, :], pacc[1][:],
                                         ACTF.Copy)
                    nc.sync.dma_start(
                        out=pool_part.ap().rearrange("k p o -> p k o"),
                        in_=pool_acc[:])
                    nc.gpsimd.collective_compute(
                        "AllReduce", OP.add,
                        replica_groups=[list(range(NC))],
                        ins=[pool_part.ap()], outs=[pool_sum.ap()])
                    pooled = cp.tile([128, 2, OUT], F32)
                    nc.sync.dma_start(
                        out=pooled[:],
                        in_=pool_sum.ap().rearrange("k p o -> p k o"))
                    pooled16 = cp.tile([128, 2, OUT], F16)
                    nc.vector.tensor_copy(pooled16[:], pooled[:])
                    fin = psB.tile([G, OUT], F32, tag="agg")
                    for kh in range(2):
                        nc.tensor.matmul(fin[:], lhsT=pooled16[:, kh, :],
                                         rhs=wfc_sb[:, kh, :],
                                         start=(kh == 0), stop=(kh == 1))
                    out_sb = cp.tile([G, OUT], F32)
                    nc.vector.tensor_tensor(out_sb[:], fin[:], bfc_sb[:], OP.add)
                    nc.sync.dma_start(out=out_d.ap(), in_=out_sb[:])

            if run_layers < NLAYERS:
                out_sb2 = cp.tile([G, OUT], F32)
                nc.vector.memset(out_sb2[:], 0.0)
                nc.sync.dma_start(out=out_d.ap(), in_=out_sb2[:])

    nc.compile()
    _BUILD_CACHE[key] = nc
    return nc


def _wrap16(vals):
    """flat [n] int array (n % 128 == 0) -> [128, n//16] int16 gather layout."""
    n = len(vals)
    out = vals.reshape(n // 16, 16).T.astype(np.int16)   # [16, n//16]
    return np.tile(out, (8, 1))


def _host_prep(inputs):
    x = np.asarray(inputs["x"], np.float32)
    ei = np.asarray(inputs["edge_index"]).astype(np.int64)
    batch = np.asarray(inputs["batch"]).astype(np.int64)
    src, dst = ei[0], ei[1]
    f16 = np.float16

    # ---- weights ----
    def pack_w(W0a, W0b, Wla, Wlb):
        w = np.zeros((128, NLAYERS, 2, 2 * HC), f16)
        w[:, 0, 0, 0:HC] = np.asarray(W0a, np.float32).astype(f16)
        w[:, 0, 0, HC:] = np.asarray(W0b, np.float32).astype(f16)
        for l in range(NLAYERS - 1):
            a = np.asarray(Wla[l], np.float32).astype(f16)
            b = np.asarray(Wlb[l], np.float32).astype(f16)
            for kh in range(2):
                w[:, l + 1, kh, 0:HC] = a[kh * 128:(kh + 1) * 128]
                w[:, l + 1, kh, HC:] = b[kh * 128:(kh + 1) * 128]
        return w

    wkv = pack_w(inputs["Wk0"], inputs["Wv0"], inputs["Wk"], inputs["Wv"])
    wqs = pack_w(inputs["Wq0"], inputs["Ws0"], inputs["Wq"], inputs["Ws"])
    wall = np.concatenate([wkv, wqs], axis=3)  # [128, L, 2, 1024]

    def pack_b(b0a, b0b, bla, blb):
        b = np.zeros((1, NLAYERS, 2 * HC), f16)
        b[0, 0, 0:HC] = np.asarray(b0a, np.float32).astype(f16)
        b[0, 0, HC:] = np.asarray(b0b, np.float32).astype(f16)
        for l in range(NLAYERS - 1):
            b[0, l + 1, 0:HC] = np.asarray(bla[l], np.float32).astype(f16)
            b[0, l + 1, HC:] = np.asarray(blb[l], np.float32).astype(f16)
        return b

    bkv = pack_b(inputs["bk0"], inputs["bv0"], inputs["bk"], inputs["bv"])
    bqs = pack_b(inputs["bq0"], inputs["bs0"], inputs["bq"], inputs["bs"])
    ball = np.concatenate([bkv, bqs], axis=2)
    use_bias = bool(np.abs(ball).max() > 0)

    wfc = np.asarray(inputs["Wfc"], np.float32).astype(f16)
    wfc_p = np.ascontiguousarray(wfc.reshape(2, 128, OUT).transpose(1, 0, 2))
    bfc_rep = np.tile(np.asarray(inputs["bfc"], np.float32)[None, :], (G, 1))
    ones1 = np.ones((1, 128), f16)

    counts = np.bincount(batch, minlength=G).astype(np.float32)
    inv_counts = (1.0 / np.maximum(counts, 1.0)).astype(np.float32)

    # ---- round-robin core assignment by degree ----
    deg = np.bincount(dst, minlength=N)
    gorder = np.argsort(-deg, kind="stable")
    core_of = np.empty(N, np.int64)
    core_of[gorder] = np.arange(N) % NC
    is_lo_node = core_of < (NC // 2)

    # per-core local ids + (lo, hi) in-degree per node
    lidx = np.empty(N, np.int64)
    nodes_c = []
    for c in range(NC):
        mine = gorder[core_of[gorder] == c]
        lidx[mine] = np.arange(len(mine))
        nodes_c.append(mine)
    dst_core = core_of[dst]
    dloc_all = lidx[dst]
    src_lo = is_lo_node[src]
    LODS, HIDS = [], []
    for c in range(NC):
        m = dst_core == c
        LODS.append(np.bincount(dloc_all[m & src_lo], minlength=NLOC))
        HIDS.append(np.bincount(dloc_all[m & ~src_lo], minlength=NLOC))

    # ---- window packing: seed profiles -> median target -> FFD with bumps ----
    def pack_seed(lod, hid):
        order = np.lexsort((-lod, -(lod + hid)))
        remaining = np.ones(len(lod), bool)
        Slo = np.zeros(W, np.int64)
        Shi = np.zeros(W, np.int64)
        for w in range(W):
            idx = order[remaining[order]]
            if len(idx) == 0:
                break
            cl = ch = 0
            cnt = 0
            take = []
            for n in idx:
                if cnt >= 128:
                    break
                if cnt == 0 or (lod[n] <= cl and hid[n] <= ch):
                    take.append(n)
                    cl = max(cl, lod[n])
                    ch = max(ch, hid[n])
                    cnt += 1
            if cnt < 128:
                taken = set(take)
                rest = np.array([n for n in idx if n not in taken])
                if len(rest):
                    inc = (np.maximum(cl, lod[rest]) - cl
                           + np.maximum(ch, hid[rest]) - ch)
                    for i in np.argsort(inc, kind="stable"):
                        if cnt >= 128:
                            break
                        n = rest[i]
                        take.append(n)
                        cl = max(cl, lod[n])
                        ch = max(ch, hid[n])
                        cnt += 1
            remaining[np.array(take)] = False
            Slo[w], Shi[w] = cl, ch
        return Slo, Shi

    profs = [pack_seed(LODS[c], HIDS[c]) for c in range(NC)]
    PL = np.array([p[0] for p in profs])
    PH = np.array([p[1] for p in profs])
    o = np.argsort(-(PL + PH), axis=1)
    PLs = np.take_along_axis(PL, o, 1)
    PHs = np.take_along_axis(PH, o, 1)
    TL = np.median(PLs, axis=0).astype(np.int64)
    TH = np.median(PHs, axis=0).astype(np.int64)

    def ffd(lod, hid, TLw, THw):
        order = np.lexsort((-lod, -(lod + hid)))
        cnt = np.zeros(W, np.int64)
        wof = np.empty(len(lod), np.int64)
        lane = np.empty(len(lod), np.int64)
        for n in order:
            feas = np.where((cnt < 128) & (TLw >= lod[n]) & (THw >= hid[n]))[0]
            if len(feas):
                j = feas[np.argmin((TLw[feas] - lod[n]) + (THw[feas] - hid[n]))]
            else:
                sp = np.where(cnt < 128)[0]
                bump = (np.maximum(lod[n] - TLw[sp], 0)
                        + np.maximum(hid[n] - THw[sp], 0))
                j = sp[np.argmin(bump)]
                TLw[j] = max(TLw[j], lod[n])
                THw[j] = max(THw[j], hid[n])
            wof[n] = j
            lane[n] = cnt[j]
            cnt[j] += 1
        return wof, lane

    # two passes: first pass grows the target; second repacks tighter
    TLw, THw = TL.copy(), TH.copy()
    assigns = []
    for _pass in range(2):
        assigns = []
        TL2, TH2 = TLw.copy(), THw.copy()
        for c in range(NC):
            wof, lane = ffd(LODS[c].copy(), HIDS[c].copy(), TL2, TH2)
            assigns.append((wof, lane))
        TLw, THw = TL2, TH2
    SL = [int(v) for v in TLw]
    SH = [int(v) for v in THw]

    # global row of every node
    row_g = np.empty(N, np.int64)
    for c in range(NC):
        wof, lane = assigns[c]
        row_g[nodes_c[c]] = c * NPAD + wof * 128 + lane

    # ---- per-core edge slots ----
    woff = np.concatenate([[0], np.cumsum(np.asarray(SL) + np.asarray(SH))])
    SLOTS = int(woff[-1])
    src_row = row_g[src]

    in_maps = []
    shared = dict(w_all=wall, b_all=ball, ones1=ones1,
                  w_fc=wfc_p, b_fc=bfc_rep.astype(np.float32))
    for c in range(NC):
        wof, lane = assigns[c]
        m = dst_core == c
        e_src_row = src_row[m]
        e_dloc = dloc_all[m]
        e_lo = src_lo[m]
        e_w = wof[e_dloc]
        e_lane = lane[e_dloc]
        # slot within (window, half, lane) group
        order = np.lexsort((e_lane, np.where(e_lo, 0, 1), e_w))
        e_src_row, e_lo, e_w, e_lane = (e_src_row[order], e_lo[order],
                                        e_w[order], e_lane[order])
        gkey = (e_w * 2 + np.where(e_lo, 0, 1)) * 128 + e_lane
        first = np.concatenate([[True], gkey[1:] != gkey[:-1]])
        gstart = np.maximum.accumulate(np.where(first, np.arange(len(gkey)), 0))
        e_slot = np.arange(len(gkey)) - gstart

        idx_flat = np.zeros(SLOTS * 128, np.int64)
        mask = np.full((128, SLOTS), MASK_NEG, np.float64)
        # lo edges
        lo_m = e_lo
        base_lo = woff[e_w]
        pos = (base_lo + e_slot) * 128 + e_lane
        idx_flat[pos[lo_m]] = e_src_row[lo_m]
        mask.reshape(-1)[((base_lo + e_slot) + e_lane * SLOTS)[lo_m]] = 0.0
        # hi edges
        hi_m = ~e_lo
        base_hi = woff[e_w] + np.asarray(SL)[e_w]
        pos = (base_hi + e_slot) * 128 + e_lane
        idx_flat[pos[hi_m]] = e_src_row[hi_m] - HALF
        mask.reshape(-1)[((base_hi + e_slot) + e_lane * SLOTS)[hi_m]] = 0.0
        assert idx_flat.min() >= 0 and idx_flat.max() < HALF

        idx16 = _wrap16(idx_flat)

        # xT with permuted rows
        xT = np.zeros((128, NPAD), f16)
        mine = nodes_c[c]
        xT[:, row_g[mine] - c * NPAD] = x[mine, :].T.astype(f16)

        # graph-pool selector
        gs_flat = np.zeros((NPAD, G), np.float32)
        gn = batch[mine]
        gs_flat[row_g[mine] - c * NPAD, gn] = inv_counts[gn]
        gsel_in = np.ascontiguousarray(
            gs_flat.reshape(W, 128, G).transpose(1, 0, 2)
            .reshape(128, W * G).astype(f16))

        in_maps.append(dict(
            shared,
            xT_loc=np.ascontiguousarray(xT),
            idx_all=np.ascontiguousarray(idx16),
            maskw=np.ascontiguousarray(mask.astype(f16)),
            gsel=gsel_in))
    return in_maps, SL, SH, use_bias


def kernel(**inputs):
    global LAST_RESULT
    in_maps, SL, SH, use_bias = _host_prep(inputs)
    run_layers = int(os.environ.get("RUN_LAYERS", str(NLAYERS)))
    kv8 = bool(int(os.environ.get("KV8", "0")))
    nc = _build(SL, SH, run_layers, use_bias, kv8)
    trace = bool(int(os.environ.get("KTRACE", "0")))
    res = bass_utils.run_bass_kernel_spmd(
        nc, in_maps, core_ids=list(range(NC)), trace=trace)
    LAST_RESULT = res
    return res.results[0]["out"].astype(np.float32)


# revision 9
# speedup vs baseline: 1.2200x; 1.2200x over previous
"""Trainium2 Bass kernel for nn_EnhancedGNNTransformerEncoder (4-layer
TransformerConv GNN + mean-pool + linear head).

Sharding: destination nodes (and their incident edges) are distributed
round-robin by degree across the 8 NeuronCores; the small weight matrices are
replicated.  Per layer each core computes K/V (fp8) + Q/skip (fp16) rows for
its own shard, the fp8 K|V rows are exchanged with an AllGather, and each core
runs the per-edge attention for its own destination windows.

Edge layout is *lane-aligned*: window w holds 128 destination nodes, one per
SBUF partition lane; slot s of lane p holds the s-th incoming edge of that
lane's node (split into a lo/hi region by source half for int16 gather
indices).  Per-edge K|V rows are fetched with dma_gather; q / softmax /
weighted aggregation then need no per-edge index work at all: q is a
contiguous row load and the segment sum over slots is an identity matmul
accumulated in PSUM.
"""

import os
import sys
import types

import numpy as np

# ---------------------------------------------------------------------------
# NTFF profile hook (absent antenv.axon_hooks on this image) so trace=True
# works under axon.
if "antenv.axon_hooks" not in sys.modules:
    _m = types.ModuleType("antenv.axon_hooks")
    _m._hook = None

    def _set(h):
        _m._hook = h

    def _get():
        return _m._hook

    _m.set_axon_ntff_profile_hook = _set
    _m.get_axon_ntff_profile_hook = _get
    sys.modules["antenv.axon_hooks"] = _m
    try:
        import antenv

        antenv.axon_hooks = _m
    except Exception:
        pass
    try:
        from trn_agent_boot.trn_boot import _ntff_profile_via_ctypes

        _m._hook = _ntff_profile_via_ctypes("/opt/axon/libaxon_pjrt.so")
    except Exception:
        pass

import concourse.bass as bass
import concourse.mybir as mybir
import concourse.tile as tile
from concourse import bacc
from concourse import bass_utils
from concourse.masks import make_identity

F16 = mybir.dt.float16
BF16 = mybir.dt.bfloat16
F32 = mybir.dt.float32
F8 = mybir.dt.float8e4
I16 = mybir.dt.int16
AX = mybir.AxisListType
OP = mybir.AluOpType
ACTF = mybir.ActivationFunctionType

# problem constants (hardcoded per the harness contract)
N, E, IN, H, C, G, OUT = 50000, 800000, 128, 8, 32, 64, 64
HC = H * C  # 256
NLAYERS = 4
NC = 8
NLOC = N // NC          # 6250
W = 49                  # windows of 128 dst lanes per core
NPAD = W * 128          # 6272 local rows (incl dummies)
NPAD_ALL = NC * NPAD    # 50176 kv rows
HALF = NPAD_ALL // 2    # 25088 (int16-addressable half; cores 0-3)
SCALE = float(1.0 / np.sqrt(C))
MASK_NEG = -30000.0
SCHUNK = 16             # slots per processing chunk

_BUILD_CACHE = {}
LAST_RESULT = None


def _build(SL, SH, run_layers, use_bias, kv8):
    """Build + compile the SPMD program.  SL/SH: per-window lo/hi slot counts
    (uniform across cores)."""
    key = (tuple(SL), tuple(SH), run_layers, use_bias, kv8, int(os.environ.get('KSCHUNK', str(SCHUNK))))
    if key in _BUILD_CACHE:
        return _BUILD_CACHE[key]

    SLOTS = int(sum(SL) + sum(SH))      # total kv slots per core
    IDXC = SLOTS * 8                    # idx columns (128 i16 per slot = 8 cols)
    KVD = F8 if kv8 else F16
    nwin = int(os.environ.get("KWIN", str(W)))
    schunk = int(os.environ.get("KSCHUNK", str(SCHUNK)))

    # per-window chunk lists: (half, slot_off_in_window, csz)
    chunks = []
    for w in range(W):
        ch = []
        for a in range(0, SL[w], schunk):
            ch.append((0, a, min(schunk, SL[w] - a)))
        for a in range(0, SH[w], schunk):
            ch.append((1, a, min(schunk, SH[w] - a)))
        chunks.append(ch)
    woff = np.concatenate([[0], np.cumsum(np.asarray(SL) + np.asarray(SH))])

    nc = bacc.Bacc("TRN2", target_bir_lowering=False, debug=False,
                   enable_asserts=False, num_devices=NC,
                   num_swdge_queues=4)

    # ---- external inputs (per-core content, same shapes) ----
    xT_loc = nc.dram_tensor("xT_loc", [128, NPAD], F16, kind="ExternalInput")
    w_all = nc.dram_tensor("w_all", [128, NLAYERS, 2, 4 * HC], F16, kind="ExternalInput")
    b_all = nc.dram_tensor("b_all", [1, NLAYERS, 4 * HC], F16, kind="ExternalInput")
    ones1 = nc.dram_tensor("ones1", [1, 128], F16, kind="ExternalInput")
    w_fc = nc.dram_tensor("w_fc", [128, 2, OUT], F16, kind="ExternalInput")
    b_fc = nc.dram_tensor("b_fc", [G, OUT], F32, kind="ExternalInput")
    idx_all = nc.dram_tensor("idx_all", [128, IDXC], I16, kind="ExternalInput")
    maskw = nc.dram_tensor("maskw", [128, SLOTS], F16, kind="ExternalInput")
    gsel = nc.dram_tensor("gsel", [128, W * G], F16, kind="ExternalInput")

    out_d = nc.dram_tensor("out", [G, OUT], F32, kind="ExternalOutput")

    # ---- internal DRAM (ping-pong by layer parity) ----
    kv_loc = [nc.dram_tensor(f"kv_loc{i}", [NPAD, 2 * HC], KVD, kind="Internal")
              for i in range(2)]
    kv_full = [nc.dram_tensor(f"kv_full{i}", [NPAD_ALL, 2 * HC], KVD,
                              kind="Internal", addr_space="Shared")
               for i in range(2)]
    qskip = [nc.dram_tensor(f"qskip{i}", [NPAD, 2 * HC], F16, kind="Internal")
             for i in range(2)]
    pool_part = nc.dram_tensor("pool_part", [2, 128, OUT], F32, kind="Internal")
    pool_sum = nc.dram_tensor("pool_sum", [2, 128, OUT], F32, kind="Internal",
                              addr_space="Shared")

    GRP = 7              # node-tiles per phase-A store group (49 = 7*7)
    NGRP = W // GRP

    with tile.TileContext(nc) as tc:
        with tc.tile_pool(name="const", bufs=1) as cp, \
             tc.tile_pool(name="kvb", bufs=2) as kvbp, \
             tc.tile_pool(name="win", bufs=2) as winp, \
             tc.tile_pool(name="psA", bufs=1, space="PSUM") as psA, \
             tc.tile_pool(name="psB", bufs=3, space="PSUM") as psB, \
             tc.tile_pool(name="psT", bufs=2, space="PSUM") as psT:

            # ---- load constants ----
            wall_sb = cp.tile([128, NLAYERS, 2, 4 * HC], F16)
            ball_sb = cp.tile([1, NLAYERS, 4 * HC], F16)
            ones_sb = cp.tile([1, 128], F16)
            wfc_sb = cp.tile([128, 2, OUT], F16)
            bfc_sb = cp.tile([G, OUT], F32)
            mask_sb = cp.tile([128, SLOTS], F16)
            gsel_sb = cp.tile([128, W * G], F16)
            ident16 = cp.tile([128, 128], F16)
            identbf = cp.tile([128, 128], BF16)
            pool_acc = cp.tile([128, 2, OUT], F32)
            hT_sb = cp.tile([128, 2, NPAD], F16)

            for t, d in [(wall_sb, w_all), (ball_sb, b_all),
                         (ones_sb, ones1), (wfc_sb, w_fc),
                         (bfc_sb, b_fc), (mask_sb, maskw), (gsel_sb, gsel)]:
                nc.sync.dma_start(out=t[:], in_=d.ap())
            make_identity(nc, ident16[:])
            make_identity(nc, identbf[:])
            nc.sync.dma_start(out=hT_sb[:, 0, :], in_=xT_loc.ap())

            qctr = [0]

            def a_group(layer, g):
                """Phase A for node tiles [g*GRP, (g+1)*GRP): K|V + Q|S rows."""
                KH = 1 if layer == 0 else 2
                par = layer % 2
                kvb = kvbp.tile([128, GRP, 2 * HC], KVD, tag="kvb")
                qsb = kvbp.tile([128, GRP, 2 * HC], F16, tag="qsb")
                for jg in range(GRP):
                    j = g * GRP + jg
                    ps = psA.tile([128, 4 * HC], F32, tag="psA")
                    for hf in range(2):
                        cs = slice(hf * 2 * HC, (hf + 1) * 2 * HC)
                        for kh in range(KH):
                            nc.tensor.matmul(
                                ps[:, cs],
                                lhsT=hT_sb[:, kh, j * 128:(j + 1) * 128],
                                rhs=wall_sb[:, layer, kh, cs],
                                start=(kh == 0),
                                stop=(not use_bias and kh == KH - 1),
                                skip_group_check=True)
                        if use_bias:
                            nc.tensor.matmul(
                                ps[:, cs], lhsT=ones_sb[:],
                                rhs=ball_sb[:, layer, cs],
                                start=False, stop=True,
                                skip_group_check=True)
                    if jg % 2 == 0:
                        nc.vector.tensor_copy(kvb[:, jg, :], ps[:, 0:2 * HC])
                        nc.scalar.activation(qsb[:, jg, :],
                                             ps[:, 2 * HC:4 * HC], ACTF.Copy)
                    else:
                        nc.scalar.activation(kvb[:, jg, :],
                                             ps[:, 0:2 * HC], ACTF.Copy)
                        nc.vector.tensor_copy(qsb[:, jg, :],
                                              ps[:, 2 * HC:4 * HC])
                base = g * GRP * 128
                dst_ap = kv_loc[par].ap()[base:base + GRP * 128, :]
                nc.sync.dma_start(
                    out=dst_ap.rearrange("(t p) e -> p t e", p=128),
                    in_=kvb[:])
                dst_ap = qskip[par].ap()[base:base + GRP * 128, :]
                nc.sync.dma_start(
                    out=dst_ap.rearrange("(t p) e -> p t e", p=128),
                    in_=qsb[:])

            def fire_collective(layer):
                par = layer % 2
                nc.gpsimd.collective_compute(
                    "AllGather", OP.bypass,
                    replica_groups=[list(range(NC))],
                    ins=[kv_loc[par].ap()], outs=[kv_full[par].ap()])

            # prologue: layer-0 phase A + collective
            for g in range(NGRP):
                a_group(0, g)
            fire_collective(0)

            for layer in range(run_layers):
                last = layer == NLAYERS - 1
                par = layer % 2

                # ===== Phase B: lane-aligned windows =====
                if last:
                    pacc = [psT.tile([128, OUT], F32, tag="trp",
                                     name=f"pacc{kh}") for kh in range(2)]
                for w in range(nwin):
                    S = SL[w] + SH[w]
                    wchunks = chunks[w]
                    idxw = winp.tile([128, S * 8], I16, tag="idxw", bufs=4)
                    nc.sync.dma_start(
                        out=idxw[:],
                        in_=idx_all.ap()[:, woff[w] * 8:(woff[w] + S) * 8])
                    qs_w = winp.tile([128, 2 * HC], F16, tag="qsw", bufs=3)
                    nc.sync.dma_start(
                        out=qs_w[:],
                        in_=qskip[par].ap()[w * 128:(w + 1) * 128, :])
                    agg = psB.tile([128, HC + H], F32, tag="agg")

                    for ci, (half, a, csz) in enumerate(wchunks):
                        # slot offset of this chunk within the window
                        soff = a if half == 0 else SL[w] + a
                        kv8_t = winp.tile([128, schunk, 2 * HC], KVD,
                                          tag="kv8", bufs=4)
                        in_ap = (kv_full[par].ap()[0:HALF, :] if half == 0
                                 else kv_full[par].ap()[HALF:NPAD_ALL, :])
                        for b in range(0, csz, 8):
                            bsz = min(8, csz - b)
                            nc.gpsimd.dma_gather(
                                out_ap=kv8_t[:, b:b + bsz, :],
                                in_ap=in_ap,
                                idxs_ap=idxw[:, (soff + b) * 8:
                                             (soff + b + bsz) * 8],
                                num_idxs=bsz * 128, num_idxs_reg=bsz * 128,
                                elem_size=2 * HC, single_packet=True,
                                queue_num=qctr[0] % 4)
                            qctr[0] += 1
                        if kv8:
                            kvb_t = winp.tile([128, schunk, 2 * HC], F16,
                                              tag="kvc", bufs=2)
                            nc.scalar.activation(kvb_t[:, 0:csz, :],
                                                 kv8_t[:, 0:csz, :], ACTF.Copy)
                        else:
                            kvb_t = kv8_t
                        # logits
                        qk = winp.tile([128, schunk, HC], F16, tag="qkpx", bufs=2)
                        nc.vector.tensor_tensor(
                            qk[:, 0:csz, :], kvb_t[:, 0:csz, 0:HC],
                            qs_w[:, None, 0:HC].to_broadcast((128, csz, HC)),
                            OP.mult)
                        qk2 = winp.tile([128, schunk * H, C // 2], F16,
                                        tag="qk2", bufs=3)
                        qkv = qk[:, 0:csz, :].rearrange(
                            "p s (h a c) -> p (s h) a c", a=2, c=C // 2)
                        nc.vector.tensor_tensor(
                            qk2[:, 0:csz * H, :], qkv[:, :, 0, :],
                            qkv[:, :, 1, :], OP.add)
                        lg = winp.tile([128, schunk * H], F16, tag="lg", bufs=3)
                        with nc.allow_low_precision(reason="logits f16 for 2x DVE"):
                            nc.vector.tensor_reduce(
                                lg[:, 0:csz * H],
                                qk2[:, 0:csz * H, :],
                                axis=AX.X, op=OP.add)
                        ml = winp.tile([128, schunk * H], F16, tag="ml", bufs=3)
                        nc.vector.scalar_tensor_tensor(
                            out=ml[:, 0:csz * H].rearrange(
                                "p (s h) -> p s h", h=H),
                            in0=lg[:, 0:csz * H].rearrange(
                                "p (s h) -> p s h", h=H),
                            scalar=SCALE,
                            in1=mask_sb[:, woff[w] + soff:woff[w] + soff + csz,
                                        None].to_broadcast((128, csz, H)),
                            op0=OP.mult, op1=OP.add)
                        # p = exp(ml), written into the extra H columns of wv
                        wv = winp.tile([128, schunk, HC + H], BF16, tag="wv",
                                       bufs=2)
                        nc.scalar.activation(
                            wv[:, 0:csz, HC:HC + H],
                            ml[:, 0:csz * H].rearrange("p (s h) -> p s h", h=H),
                            ACTF.Exp)
                        pX = winp.tile([128, schunk, HC], BF16, tag="qkpx",
                                       bufs=2)
                        nc.scalar.activation(
                            pX[:, 0:csz, :].rearrange(
                                "p s (h c) -> p s h c", c=C),
                            wv[:, 0:csz, HC:HC + H][:, :, :, None]
                            .to_broadcast((128, csz, H, C)),
                            ACTF.Copy)
                        nc.vector.tensor_tensor(
                            wv[:, 0:csz, 0:HC], kvb_t[:, 0:csz, HC:2 * HC],
                            pX[:, 0:csz, :], OP.mult)
                        for s in range(csz):
                            nc.tensor.matmul(
                                agg[:], lhsT=identbf[:], rhs=wv[:, s, :],
                                start=(ci == 0 and s == 0),
                                stop=(ci == len(wchunks) - 1 and s == csz - 1),
                                skip_group_check=True)

                    # ---- epilogue (once per window) ----
                    rs0 = winp.tile([128, H], F32, tag="rs0", bufs=2)
                    nc.vector.tensor_scalar_add(rs0[:], agg[:, HC:HC + H], 1e-16)
                    rs = winp.tile([128, H], F32, tag="rs", bufs=2)
                    nc.vector.reciprocal(rs[:], rs0[:])
                    tmp = winp.tile([128, HC], F32, tag="tmp", bufs=2)
                    nc.vector.tensor_tensor(
                        tmp[:].rearrange("p (h c) -> p h c", c=C),
                        agg[:, 0:HC].rearrange("p (h c) -> p h c", c=C),
                        rs[:, :, None].to_broadcast((128, H, C)),
                        OP.mult)
                    tmp2 = winp.tile([128, HC], F32, tag="tmp2", bufs=2)
                    nc.vector.tensor_tensor(tmp2[:], tmp[:], qs_w[:, HC:2 * HC],
                                            OP.add)
                    h_nm = winp.tile([128, HC], F16, tag="hnm", bufs=2)
                    nc.scalar.activation(h_nm[:], tmp2[:], ACTF.Relu)

                    if last:
                        for kh in range(2):
                            nc.tensor.matmul(
                                pacc[kh][:],
                                lhsT=h_nm[:, kh * 128:(kh + 1) * 128],
                                rhs=gsel_sb[:, w * G:(w + 1) * G],
                                start=(w == 0), stop=(w == nwin - 1),
                                skip_group_check=True)
                    else:
                        for kh in range(2):
                            trp = psT.tile([128, 128], F16, tag="trp")
                            nc.tensor.transpose(
                                trp[:], h_nm[:, kh * 128:(kh + 1) * 128],
                                ident16[:])
                            nc.scalar.activation(
                                hT_sb[:, kh, w * 128:(w + 1) * 128], trp[:],
                                ACTF.Copy)

                    # interleave next layer's phase A + collective
                    if not last:
                        if w % GRP == GRP - 1 and w // GRP < NGRP:
                            a_group(layer + 1, w // GRP)
                        if w == nwin - 1:
                            fire_collective(layer + 1)

                # ===== Phase C =====
                if last:
                    nc.vector.tensor_copy(pool_acc[:, 0, :], pacc[0][:])
                    nc.scalar.activation(pool_acc[:, 1, :], pacc[1][:],
                                         ACTF.Copy)
                    nc.sync.dma_start(
                        out=pool_part.ap().rearrange("k p o -> p k o"),
                        in_=pool_acc[:])
                    nc.gpsimd.collective_compute(
                        "AllReduce", OP.add,
                        replica_groups=[list(range(NC))],
                        ins=[pool_part.ap()], outs=[pool_sum.ap()])
                    pooled = cp.tile([128, 2, OUT], F32)
                    nc.sync.dma_start(
                        out=pooled[:],
                        in_=pool_sum.ap().rearrange("k p o -> p k o"))
                    pooled16 = cp.tile([128, 2, OUT], F16)
                    nc.vector.tensor_copy(pooled16[:], pooled[:])
                    fin = psB.tile([G, OUT], F32, tag="agg")
                    for kh in range(2):
                        nc.tensor.matmul(fin[:], lhsT=pooled16[:, kh, :],
                                         rhs=wfc_sb[:, kh, :],
                                         start=(kh == 0), stop=(kh == 1))
                    out_sb = cp.tile([G, OUT], F32)
                    nc.vector.tensor_tensor(out_sb[:], fin[:], bfc_sb[:], OP.add)
                    nc.sync.dma_start(out=out_d.ap(), in_=out_sb[:])

            if run_layers < NLAYERS:
                out_sb2 = cp.tile([G, OUT], F32)
                nc.vector.memset(out_sb2[:], 0.0)
                nc.sync.dma_start(out=out_d.ap(), in_=out_sb2[:])

    nc.compile()
    _BUILD_CACHE[key] = nc
    return nc


def _wrap16(vals):
    """flat [n] int array (n % 128 == 0) -> [128, n//16] int16 gather layout."""
    n = len(vals)
    out = vals.reshape(n // 16, 16).T.astype(np.int16)   # [16, n//16]
    return np.tile(out, (8, 1))


def _host_prep(inputs):
    x = np.asarray(inputs["x"], np.float32)
    ei = np.asarray(inputs["edge_index"]).astype(np.int64)
    batch = np.asarray(inputs["batch"]).astype(np.int64)
    src, dst = ei[0], ei[1]
    f16 = np.float16

    # ---- weights ----
    def pack_w(W0a, W0b, Wla, Wlb):
        w = np.zeros((128, NLAYERS, 2, 2 * HC), f16)
        w[:, 0, 0, 0:HC] = np.asarray(W0a, np.float32).astype(f16)
        w[:, 0, 0, HC:] = np.asarray(W0b, np.float32).astype(f16)
        for l in range(NLAYERS - 1):
            a = np.asarray(Wla[l], np.float32).astype(f16)
            b = np.asarray(Wlb[l], np.float32).astype(f16)
            for kh in range(2):
                w[:, l + 1, kh, 0:HC] = a[kh * 128:(kh + 1) * 128]
                w[:, l + 1, kh, HC:] = b[kh * 128:(kh + 1) * 128]
        return w

    wkv = pack_w(inputs["Wk0"], inputs["Wv0"], inputs["Wk"], inputs["Wv"])
    wqs = pack_w(inputs["Wq0"], inputs["Ws0"], inputs["Wq"], inputs["Ws"])
    wall = np.concatenate([wkv, wqs], axis=3)  # [128, L, 2, 1024]

    def pack_b(b0a, b0b, bla, blb):
        b = np.zeros((1, NLAYERS, 2 * HC), f16)
        b[0, 0, 0:HC] = np.asarray(b0a, np.float32).astype(f16)
        b[0, 0, HC:] = np.asarray(b0b, np.float32).astype(f16)
        for l in range(NLAYERS - 1):
            b[0, l + 1, 0:HC] = np.asarray(bla[l], np.float32).astype(f16)
            b[0, l + 1, HC:] = np.asarray(blb[l], np.float32).astype(f16)
        return b

    bkv = pack_b(inputs["bk0"], inputs["bv0"], inputs["bk"], inputs["bv"])
    bqs = pack_b(inputs["bq0"], inputs["bs0"], inputs["bq"], inputs["bs"])
    ball = np.concatenate([bkv, bqs], axis=2)
    use_bias = bool(np.abs(ball).max() > 0)

    wfc = np.asarray(inputs["Wfc"], np.float32).astype(f16)
    wfc_p = np.ascontiguousarray(wfc.reshape(2, 128, OUT).transpose(1, 0, 2))
    bfc_rep = np.tile(np.asarray(inputs["bfc"], np.float32)[None, :], (G, 1))
    ones1 = np.ones((1, 128), f16)

    counts = np.bincount(batch, minlength=G).astype(np.float32)
    inv_counts = (1.0 / np.maximum(counts, 1.0)).astype(np.float32)

    # ---- round-robin core assignment by degree ----
    deg = np.bincount(dst, minlength=N)
    gorder = np.argsort(-deg, kind="stable")
    core_of = np.empty(N, np.int64)
    core_of[gorder] = np.arange(N) % NC
    is_lo_node = core_of < (NC // 2)

    # per-core local ids + (lo, hi) in-degree per node
    lidx = np.empty(N, np.int64)
    nodes_c = []
    for c in range(NC):
        mine = gorder[core_of[gorder] == c]
        lidx[mine] = np.arange(len(mine))
        nodes_c.append(mine)
    dst_core = core_of[dst]
    dloc_all = lidx[dst]
    src_lo = is_lo_node[src]
    LODS, HIDS = [], []
    for c in range(NC):
        m = dst_core == c
        LODS.append(np.bincount(dloc_all[m & src_lo], minlength=NLOC))
        HIDS.append(np.bincount(dloc_all[m & ~src_lo], minlength=NLOC))

    # ---- window packing: seed profiles -> median target -> FFD with bumps ----
    def pack_seed(lod, hid):
        order = np.lexsort((-lod, -(lod + hid)))
        remaining = np.ones(len(lod), bool)
        Slo = np.zeros(W, np.int64)
        Shi = np.zeros(W, np.int64)
        for w in range(W):
            idx = order[remaining[order]]
            if len(idx) == 0:
                break
            cl = ch = 0
            cnt = 0
            take = []
            for n in idx:
                if cnt >= 128:
                    break
                if cnt == 0 or (lod[n] <= cl and hid[n] <= ch):
                    take.append(n)
                    cl = max(cl, lod[n])
                    ch = max(ch, hid[n])
                    cnt += 1
            if cnt < 128:
                taken = set(take)
                rest = np.array([n for n in idx if n not in taken])
                if len(rest):
                    inc = (np.maximum(cl, lod[rest]) - cl
                           + np.maximum(ch, hid[rest]) - ch)
                    for i in np.argsort(inc, kind="stable"):
                        if cnt >= 128:
                            break
                        n = rest[i]
                        take.append(n)
                        cl = max(cl, lod[n])
                        ch = max(ch, hid[n])
                        cnt += 1
            remaining[np.array(take)] = False
            Slo[w], Shi[w] = cl, ch
        return Slo, Shi

    profs = [pack_seed(LODS[c], HIDS[c]) for c in range(NC)]
    PL = np.array([p[0] for p in profs])
    PH = np.array([p[1] for p in profs])
    o = np.argsort(-(PL + PH), axis=1)
    PLs = np.take_along_axis(PL, o, 1)
    PHs = np.take_along_axis(PH, o, 1)
    TL = np.median(PLs, axis=0).astype(np.int64)
    TH = np.median(PHs, axis=0).astype(np.int64)

    def ffd(lod, hid, TLw, THw):
        order = np.lexsort((-lod, -(lod + hid)))
        cnt = np.zeros(W, np.int64)
        wof = np.empty(len(lod), np.int64)
        lane = np.empty(len(lod), np.int64)
        for n in order:
            feas = np.where((cnt < 128) & (TLw >= lod[n]) & (THw >= hid[n]))[0]
            if len(feas):
                j = feas[np.argmin((TLw[feas] - lod[n]) + (THw[feas] - hid[n]))]
            else:
                sp = np.where(cnt < 128)[0]
                bump = (np.maximum(lod[n] - TLw[sp], 0)
                        + np.maximum(hid[n] - THw[sp], 0))
                j = sp[np.argmin(bump)]
                TLw[j] = max(TLw[j], lod[n])
                THw[j] = max(THw[j], hid[n])
            wof[n] = j
            lane[n] = cnt[j]
            cnt[j] += 1
        return wof, lane

    # two passes: first pass grows the target; second repacks tighter
    TLw, THw = TL.copy(), TH.copy()
    assigns = []
    for _pass in range(2):
        assigns = []
        TL2, TH2 = TLw.copy(), THw.copy()
        for c in range(NC):
            wof, lane = ffd(LODS[c].copy(), HIDS[c].copy(), TL2, TH2)
            assigns.append((wof, lane))
        TLw, THw = TL2, TH2
    SL = [int(v) for v in TLw]
    SH = [int(v) for v in THw]

    # global row of every node
    row_g = np.empty(N, np.int64)
    for c in range(NC):
        wof, lane = assigns[c]
        row_g[nodes_c[c]] = c * NPAD + wof * 128 + lane

    # ---- per-core edge slots ----
    woff = np.concatenate([[0], np.cumsum(np.asarray(SL) + np.asarray(SH))])
    SLOTS = int(woff[-1])
    src_row = row_g[src]

    in_maps = []
    shared = dict(w_all=wall, b_all=ball, ones1=ones1,
                  w_fc=wfc_p, b_fc=bfc_rep.astype(np.float32))
    for c in range(NC):
        wof, lane = assigns[c]
        m = dst_core == c
        e_src_row = src_row[m]
        e_dloc = dloc_all[m]
        e_lo = src_lo[m]
        e_w = wof[e_dloc]
        e_lane = lane[e_dloc]
        # slot within (window, half, lane) group
        order = np.lexsort((e_lane, np.where(e_lo, 0, 1), e_w))
        e_src_row, e_lo, e_w, e_lane = (e_src_row[order], e_lo[order],
                                        e_w[order], e_lane[order])
        gkey = (e_w * 2 + np.where(e_lo, 0, 1)) * 128 + e_lane
        first = np.concatenate([[True], gkey[1:] != gkey[:-1]])
        gstart = np.maximum.accumulate(np.where(first, np.arange(len(gkey)), 0))
        e_slot = np.arange(len(gkey)) - gstart

        idx_flat = np.zeros(SLOTS * 128, np.int64)
        mask = np.full((128, SLOTS), MASK_NEG, np.float64)
        # lo edges
        lo_m = e_lo
        base_lo = woff[e_w]
        pos = (base_lo + e_slot) * 128 + e_lane
        idx_flat[pos[lo_m]] = e_src_row[lo_m]
        mask.reshape(-1)[((base_lo + e_slot) + e_lane * SLOTS)[lo_m]] = 0.0
        # hi edges
        hi_m = ~e_lo
        base_hi = woff[e_w] + np.asarray(SL)[e_w]
        pos = (base_hi + e_slot) * 128 + e_lane
        idx_flat[pos[hi_m]] = e_src_row[hi_m] - HALF
        mask.reshape(-1)[((base_hi + e_slot) + e_lane * SLOTS)[hi_m]] = 0.0
        assert idx_flat.min() >= 0 and idx_flat.max() < HALF

        idx16 = _wrap16(idx_flat)

        # xT with permuted rows
        xT = np.zeros((128, NPAD), f16)
        mine = nodes_c[c]
        xT[:, row_g[mine] - c * NPAD] = x[mine, :].T.astype(f16)

        # graph-pool selector
        gs_flat = np.zeros((NPAD, G), np.float32)
        gn = batch[mine]
        gs_flat[row_g[mine] - c * NPAD, gn] = inv_counts[gn]
        gsel_in = np.ascontiguousarray(
            gs_flat.reshape(W, 128, G).transpose(1, 0, 2)
            .reshape(128, W * G).astype(f16))

        in_maps.append(dict(
            shared,
            xT_loc=np.ascontiguousarray(xT),
            idx_all=np.ascontiguousarray(idx16),
            maskw=np.ascontiguousarray(mask.astype(f16)),
            gsel=gsel_in))
    return in_maps, SL, SH, use_bias


def kernel(**inputs):
    global LAST_RESULT
    in_maps, SL, SH, use_bias = _host_prep(inputs)
    run_layers = int(os.environ.get("RUN_LAYERS", str(NLAYERS)))
    kv8 = bool(int(os.environ.get("KV8", "1")))
    nc = _build(SL, SH, run_layers, use_bias, kv8)
    trace = bool(int(os.environ.get("KTRACE", "0")))
    res = bass_utils.run_bass_kernel_spmd(
        nc, in_maps, core_ids=list(range(NC)), trace=trace)
    LAST_RESULT = res
    return res.results[0]["out"].astype(np.float32)
